# revision 1
# baseline (speedup 1.0000x reference)
"""Trainium2 Bass kernel for DCRNN-Temporal (gnn_message_passing).

Contract: kernel(**inputs) takes FULL numpy inputs (x, edge_index,
edge_weight, w_z, b_z, w_r, b_r, w_h, b_h, w_lin, b_lin) and returns the
FULL [N, 12] output, running a Bass SPMD kernel on 8 NeuronCores.

Math (H0 = 0 simplifies the DCRNN cell):
  R is unused (H0*R = 0), both remaining DConvs share the same diffusion
  features. With per-edge weights 1/deg(src) folded into pre-scaled
  tables and the Chebyshev recurrence folded into effective weights:
    T1o = P_f x, T1i = P_r x, Y2o = P_f T1o, Y2i = P_r T1i
    G   = [x | T1o, Y2o, T1i, Y2i] @ Weff + beff    (z | h gates)
    H   = sigmoid(-Gz - bz) * tanh(Gh + bh)         (= (1-Z)*Htilde)
    out = relu(H) @ w_lin + b_lin

Sharding: nodes partitioned by destination across 8 cores.  Each
propagate = dma_gather row gathers (256B rows) + DVE adds over
degree-sorted ELL rounds (scatter-free).  dma_gather indices are int16,
so every node table uses one global 6256-periodic layout (6250 node
rows + 6 zero rows per core; gid(v) = (v//6250)*6256 + v%6250 < 50048)
and each direction's edges split into two windowed streams:
  A: gid < 32768 (idx = gid),  B: idx = gid - 32768 (table AP offset).

Performance structure (v2): SWDGE descriptor generation is the
bottleneck (~7.8ns/idx serialized per queue).  The ucode runs 4 SWDGE
queues that generate concurrently, so every gather/scatter chunk is
issued round-robin across queue 0..3 with a deep tile pool (measured
~2.0us per 896-idx call vs 7.6us single-queue).  Each stream scatters
into its OWN DRAM buffer (A/B concurrent on different queues); the A+B
merge happens during the reload via one DVE add.  The two AllGathers
(one per direction) are issued mid-way through the NEXT hop's gather
stream so their transfer hides under SWDGE work.
"""

import os
import sys

for _p in ("/opt/trn_rl_repo", "/root/.axon_site/_ro/trn_rl_repo"):
    if os.path.isdir(_p) and _p not in sys.path:
        sys.path.insert(0, _p)
        break

import numpy as np

import concourse.bass as bass
import concourse.tile as tile
from concourse import bacc, mybir
from concourse import bass_utils
from concourse.masks import make_identity

F = 32          # node feature dim
FO = 64         # out channels per gate
GATES = 2 * FO
PER = 12        # head output dim
NCORES = 8
ES = 64         # table row length (f32) = 256B, required by dma_gather
CH = 7          # SWDGE chunk width in 128-cols (<=896 descriptors/call)
NT = 512        # node tile (free dim) for gate/head matmuls
WINDOW = 32768  # int16 idx window
GAPZ = 6        # zero rows appended per core in the global table layout
NQ = 4          # SWDGE queues (ucode max); round-robin for parallel gen

FP32 = mybir.dt.float32
I16 = mybir.dt.int16


def _ceil(a, b):
    return -(-a // b)


class _StageDone(Exception):
    pass


def _wrap16(idx_seq):
    """[n] -> [128, n//16] int16, wrapped in 16 partitions, replicated 8x."""
    w = idx_seq.reshape(-1, 16).T.astype(np.int16)
    return np.ascontiguousarray(np.tile(w, (8, 1)))


# ----------------------------------------------------------------------------
# Host-side graph preprocessing
# ----------------------------------------------------------------------------

def _build_streams(dst, src_gid, N, NLOC, P, W):
    """ELL-round structures for one directed edge set, split into
    idx-window streams A (gid < W) and B (gid >= W)."""
    core = dst // NLOC
    dl = dst - core * NLOC
    streams = []
    for s, mask in (("A", src_gid < W), ("B", src_gid >= W)):
        percore = []
        for p in range(P):
            sel = np.nonzero((core == p) & mask)[0]
            cnt = np.bincount(dl[sel], minlength=NLOC)
            perm = np.argsort(-cnt, kind="stable").astype(np.int32)
            rank = np.empty(NLOC, np.int32)
            rank[perm] = np.arange(NLOC, dtype=np.int32)
            order = np.argsort(dl[sel], kind="stable")
            es = sel[order]
            ptr = np.zeros(NLOC + 1, np.int64)
            np.cumsum(cnt, out=ptr[1:])
            j_arr = np.arange(len(es), dtype=np.int64) - ptr[dl[es]]
            percore.append(dict(es=es, j=j_arr, r=rank[dl[es]].astype(np.int64),
                                cnt=cnt, perm=perm))
        maxdeg = int(max(pc["cnt"].max(initial=0) for pc in percore))
        NJ = []
        for j in range(maxdeg):
            nj = max(int((pc["cnt"] > j).sum()) for pc in percore)
            NJ.append(_ceil(nj, 128) * 128)
        off = np.zeros(len(NJ) + 1, np.int64)
        np.cumsum(NJ, out=off[1:])
        EP = int(off[-1])
        for pc in percore:
            pc["slot"] = off[pc["j"]] + pc["r"]
        streams.append(dict(name=s, percore=percore, NJ=NJ, off=off, EP=EP))
    return streams


def preprocess(x, edge_index, edge_weight, w_z, b_z, w_r, b_r, w_h, b_h,
               w_lin, b_lin, P=NCORES, window=WINDOW):
    N, Fx = x.shape
    assert Fx == F
    assert N % P == 0
    NLOC = N // P
    NLOCP = _ceil(NLOC, 128) * 128
    CACC = NLOCP // 128
    NL6 = NLOC + GAPZ
    NTAB = P * NL6
    W = min(window, NTAB)
    assert NTAB - W < 32768 and NLOC < W

    row = np.asarray(edge_index[0], dtype=np.int64)
    col = np.asarray(edge_index[1], dtype=np.int64)
    ew = np.asarray(edge_weight, dtype=np.float64)
    deg_out = np.bincount(row, weights=ew, minlength=N)
    deg_in = np.bincount(col, weights=ew, minlength=N)
    with np.errstate(divide="ignore"):
        doi = np.where(deg_out > 0, 1.0 / deg_out, 0.0).astype(np.float32)
        dii = np.where(deg_in > 0, 1.0 / deg_in, 0.0).astype(np.float32)

    gid = (np.arange(N) // NLOC) * NL6 + (np.arange(N) % NLOC)
    xf = np.asarray(x, np.float32)

    def table(scaled):
        t = np.zeros((NTAB, ES), np.float32)
        t[gid, :F] = scaled
        return t

    XF = table(xf * doi[:, None])
    XR = table(xf * dii[:, None])

    # pad slot targets: a zero row inside each window
    apad = NLOC  # core-0 zero row, < W
    zq = _ceil(W - NLOC, NL6)  # first core whose zero row is >= W
    bpad = zq * NL6 + NLOC - W
    assert 0 <= bpad < NTAB - W or W == NTAB

    gsrcF = gid[row]
    gsrcR = gid[col]
    stF = _build_streams(col, gsrcF, N, NLOC, P, W)
    stR = _build_streams(row, gsrcR, N, NLOC, P, W)

    # effective gate weights (K = 3); WA rows = [T1o, T1i, Y2o, Y2i] so the
    # T1 half [0:2F] can matmul early (during hop 2) with contiguous lhsT
    assert w_z.shape[1] == 3

    def gate_w(w):
        w = np.asarray(w, np.float32)
        wx = w[0, 0, :F] + w[1, 0, :F] - w[0, 2, :F] - w[1, 2, :F]
        wa = np.concatenate(
            [w[0, 1, :F], w[1, 1, :F], 2.0 * w[0, 2, :F], 2.0 * w[1, 2, :F]], 0)
        return wx, wa

    wxz, waz = gate_w(w_z)
    wxh, wah = gate_w(w_h)
    WA = np.ascontiguousarray(np.concatenate([waz, wah], axis=1))
    WX = np.ascontiguousarray(np.concatenate([wxz, wxh], axis=1))
    biasS = np.ascontiguousarray(-np.asarray(b_z, np.float32)[:, None])
    biasT = np.ascontiguousarray(np.asarray(b_h, np.float32)[:, None])
    WL = np.asarray(w_lin, np.float32)
    BL = np.ascontiguousarray(np.asarray(b_lin, np.float32)[:, None])

    def expand(vec):  # [NLOCP] -> [128, CACC, F] accumulator-layout expand
        return np.ascontiguousarray(
            np.repeat(vec.reshape(CACC, 128).T, F, axis=1)
        ).reshape(128, CACC, F)

    in_maps = []
    for p in range(P):
        m = {"xf": XF, "xr": XR, "wa": WA, "wx": WX, "biass": biasS,
             "biast": biasT, "wl": WL, "bl": BL}
        for dname, st, gsrc, srcv in (("f", stF, gsrcF, row),
                                      ("r", stR, gsrcR, col)):
            for s in st:
                pc = s["percore"][p]
                if s["EP"] == 0:
                    continue
                base, pad = (0, apad) if s["name"] == "A" else (W, bpad)
                ivals = np.full(s["EP"], pad, np.int64)
                ivals[pc["slot"]] = gsrc[pc["es"]] - base
                m[f"ix{dname}{s['name'].lower()}"] = _wrap16(ivals)
                sidx = np.full(NLOCP, NLOC, np.int64)
                sidx[:NLOC] = pc["perm"]
                m[f"sx{dname}{s['name'].lower()}"] = _wrap16(sidx)
        sl = slice(p * NLOC, (p + 1) * NLOC)
        xp = np.zeros((NLOCP, F), np.float32)
        xp[:NLOC] = xf[sl]
        m["xpt"] = np.ascontiguousarray(xp.T)
        tmp = np.zeros(NLOCP, np.float32)
        tmp[:NLOC] = doi[sl]
        m["doe"] = expand(tmp)
        tmp = np.zeros(NLOCP, np.float32)
        tmp[:NLOC] = dii[sl]
        m["die"] = expand(tmp)
        in_maps.append(m)

    meta = dict(N=N, P=P, NLOC=NLOC, NLOCP=NLOCP, CACC=CACC, NL6=NL6,
                NTAB=NTAB, W=W,
                streams={"f": [dict(name=s["name"], NJ=s["NJ"], off=s["off"],
                                    EP=s["EP"]) for s in stF],
                         "r": [dict(name=s["name"], NJ=s["NJ"], off=s["off"],
                                    EP=s["EP"]) for s in stR]})
    return in_maps, meta


# ----------------------------------------------------------------------------
# Device program
# ----------------------------------------------------------------------------

def build_program(meta, debug=False):
    from contextlib import ExitStack

    stage = int(os.environ.get("KSTAGE", "9"))
    konly = os.environ.get("KONLY", "")
    nogath = os.environ.get("KNOGATH", "")
    noadd = os.environ.get("KNOADD", "")
    noscat = os.environ.get("KNOSCAT", "")

    N, P = meta["N"], meta["P"]
    NLOC, NLOCP, CACC = meta["NLOC"], meta["NLOCP"], meta["CACC"]
    NL6, NTAB, W = meta["NL6"], meta["NTAB"], meta["W"]
    streams = meta["streams"]

    nc = bacc.Bacc("TRN2", target_bir_lowering=False, debug=False,
                   num_devices=P, num_swdge_queues=NQ)

    def din(name, shape, dt=FP32):
        return nc.dram_tensor(name, list(shape), dt, kind="ExternalInput").ap()

    def dout(name, shape, dt=FP32):
        return nc.dram_tensor(name, list(shape), dt, kind="ExternalOutput").ap()

    xf_d = din("xf", (NTAB, ES))
    xr_d = din("xr", (NTAB, ES))
    ix_d, sx_d = {}, {}
    for d in ("f", "r"):
        for s in streams[d]:
            if s["EP"] == 0:
                continue
            k = d + s["name"].lower()
            ix_d[k] = din("ix" + k, (128, s["EP"] // 16), I16)
            sx_d[k] = din("sx" + k, (128, NLOCP // 16), I16)
    xpt_d = din("xpt", (F, NLOCP))
    doe_d = din("doe", (128, CACC, F))
    die_d = din("die", (128, CACC, F))
    wa_d = din("wa", (4 * F, GATES))
    wx_d = din("wx", (F, GATES))
    biass_d = din("biass", (FO, 1))
    biast_d = din("biast", (FO, 1))
    wl_d = din("wl", (FO, PER))
    bl_d = din("bl", (PER, 1))
    out_d = dout("out", (PER, NLOCP))

    EPmax = max(s["EP"] for d in ("f", "r") for s in streams[d])
    qctr = [0]

    def nextq():
        q = qctr[0] % NQ
        qctr[0] += 1
        return q

    with tile.TileContext(nc) as tc, ExitStack() as ctx:
      try:
          sb = ctx.enter_context(tc.tile_pool(name="sb", bufs=1))
          ya = ctx.enter_context(tc.tile_pool(name="ya", bufs=3))
          gp = ctx.enter_context(tc.tile_pool(name="gp", bufs=8))
          ixp = ctx.enter_context(tc.tile_pool(name="ixp", bufs=2))
          big = ctx.enter_context(tc.tile_pool(name="big", bufs=1))
          mg = ctx.enter_context(tc.tile_pool(name="mg", bufs=2))
          px = ctx.enter_context(tc.tile_pool(name="px", bufs=3))
          pp = ctx.enter_context(tc.tile_pool(name="pp", bufs=2, space="PSUM"))
          pt = ctx.enter_context(tc.tile_pool(name="pt", bufs=2, space="PSUM"))
          dr = ctx.enter_context(tc.tile_pool(name="dr", bufs=1, space="DRAM"))

          phi = sb.tile([128, NLOCP], FP32, tag="phi")
          gPZ = sb.tile([FO, NLOCP], FP32, tag="gPZ")
          gPH = sb.tile([FO, NLOCP], FP32, tag="gPH")
          doe = sb.tile([128, CACC, F], FP32, tag="doe")
          die = sb.tile([128, CACC, F], FP32, tag="die")
          sxt = {}
          for d in ("f", "r"):
              for s in streams[d]:
                  if s["EP"] == 0:
                      continue
                  k = d + s["name"].lower()
                  sxt[k] = sb.tile([128, NLOCP // 16], I16, tag="sx" + k,
                                   name="sx" + k)
          wa = sb.tile([4 * F, GATES], FP32, tag="wa")
          wx = sb.tile([F, GATES], FP32, tag="wx")
          bS = sb.tile([FO, 1], FP32, tag="bS")
          bT = sb.tile([FO, 1], FP32, tag="bT")
          wl = sb.tile([FO, PER], FP32, tag="wl")
          bl = sb.tile([PER, 1], FP32, tag="bl")
          ident = sb.tile([128, 128], FP32, tag="ident")

          NBNC = _ceil(max(NLOCP, NL6), 128) * 128
          CB = NBNC // 128
          # per-stream scatter targets (A/B separated so their scatter_adds
          # run concurrently on different queues; merged on reload by DVE)
          bnd = {}
          for hop_i in (1, 2):
              for d in ("f", "r"):
                  for s in ("a", "b"):
                      nm = f"bnd{hop_i}{d}{s}"
                      bnd[(hop_i, d, s)] = dr.tile([NBNC, ES], FP32,
                                                   tag=nm, name=nm)
          bounS = {d: dr.tile([NBNC, ES], FP32, tag="bounS" + d,
                              name="bounS" + d) for d in ("f", "r")}
          # Shared HBM output lets the AllGather write in place (each core
          # deposits its slab) instead of the slow HBM-HBM collective path.
          ag = {d: nc.dram_tensor("agsh" + d, [NTAB, ES], FP32,
                                  addr_space="Shared").ap()
                for d in ("f", "r")}

          # --- loads --------------------------------------------------------
          for k, t in sxt.items():
              nc.sync.dma_start(t[:], sx_d[k])
          nc.sync.dma_start(doe[:], doe_d)
          nc.sync.dma_start(die[:], die_d)
          nc.sync.dma_start(wa[:], wa_d)
          nc.sync.dma_start(wx[:], wx_d)
          nc.sync.dma_start(bS[:], biass_d)
          nc.sync.dma_start(bT[:], biast_d)
          nc.sync.dma_start(wl[:], wl_d)
          nc.sync.dma_start(bl[:], bl_d)
          make_identity(nc, ident[:])

          def wrap(dram_tile, c0, cn):
              apv = dram_tile[:].rearrange("(c p) f -> p c f", p=128)
              return apv[:, :, c0:c0 + cn]

          # --- prezero DRAM scratch from a zeroed tile (scalar queue so the
          # sync queue stays free for the first index loads) ----------------
          z0 = ya.tile([128, CB, ES], FP32, tag="y", name="z0")
          nc.vector.memset(z0[:], 0.0)
          for t in bnd.values():
              nc.scalar.dma_start(wrap(t, 0, ES), z0[:])

          # --- one windowed-stream propagate --------------------------------
          def prop_stream(yt, sdesc, ixkey, table_ap, estep, hook=None):
              EP = sdesc["EP"]
              EPc = EP // 128
              roff = [int(o) // 128 for o in sdesc["off"]]
              nrounds = len(sdesc["NJ"])
              ixt = ixp.tile([128, EPmax // 16], I16, tag="ix")
              nc.sync.dma_start(ixt[:, :EP // 16], ix_d[ixkey])
              for ci, c0 in enumerate(range(0, EPc, CH)):
                  if hook is not None and ci == 18:
                      hook()
                      hook = None
                  c1 = min(c0 + CH, EPc)
                  g = gp.tile([128, CH, ES], FP32, tag="g")
                  if nogath:
                      nc.vector.memset(g[:], 0.0)
                  else:
                      nc.gpsimd.dma_gather(
                          out_ap=g[:, :c1 - c0, :],
                          in_ap=table_ap,
                          idxs_ap=ixt[:, c0 * 8:c1 * 8],
                          num_idxs=(c1 - c0) * 128,
                          num_idxs_reg=(c1 - c0) * 128,
                          elem_size=ES,
                          elem_step=estep,
                          queue_num=nextq(),
                      )
                  if noadd:
                      continue
                  for j in range(nrounds):
                      s = max(roff[j], c0)
                      e = min(roff[j + 1], c1)
                      if s >= e:
                          continue
                      ys = s - roff[j]
                      nc.vector.tensor_tensor(
                          out=yt[:, ys:ys + e - s, 0:F],
                          in0=yt[:, ys:ys + e - s, 0:F],
                          in1=g[:, s - c0:e - c0, 0:F],
                          op=mybir.AluOpType.add,
                      )

          def hop(hop_i, d, tabA, tabB, estep, hook=None):
              """Run both streams of direction d's propagate; each stream
              scatters into its own DRAM buffer right after its adds, on its
              own queue (A/B scatters run concurrently).  `hook` is injected
              a few chunks into the first stream (AllGather issue point:
              late enough that its input chain has drained, early enough to
              hide the transfer under this hop's SWDGE work)."""
              yts = []
              for s in streams[d]:
                  if s["EP"] == 0:
                      continue
                  k = d + s["name"].lower()
                  if konly and k not in konly.split(","):
                      continue
                  yt = ya.tile([128, CACC, ES], FP32, tag="y")
                  nc.vector.memset(yt[:], 0.0)
                  prop_stream(yt, s, k, tabA if s["name"] == "A" else tabB,
                              estep, hook=hook)
                  hook = None
                  yts.append((k, s, yt))
              if noscat:
                  return
              scq = [nextq() for _ in yts]
              for c0 in range(0, CACC, CH):
                  c1 = min(c0 + CH, CACC)
                  for (k, s, yt), q in zip(yts, scq):
                      tgt = bnd[(hop_i, d, s["name"].lower())]
                      nc.gpsimd.dma_scatter_add(
                          out_ap=tgt[:],
                          in_ap=yt[:, c0:c1, :],
                          idxs_ap=sxt[k][:, c0 * 8:c1 * 8],
                          num_idxs=(c1 - c0) * 128,
                          num_idxs_reg=(c1 - c0) * 128,
                          elem_size=ES,
                          queue_num=q,
                      )

          yall = big.tile([128, CACC, 4 * F], FP32, tag="big")

          def merge_scale(hop_i, d, blk, scale_t):
              """Load A+B scatter buffers, merge into yall block `blk`;
              for hop 1 also write the scaled table slab + AllGather input."""
              dst = yall[:, :, blk * F:(blk + 1) * F]
              nc.sync.dma_start(dst,
                                wrap(bnd[(hop_i, d, "a")], 0, F)[:, :CACC, :])
              tmp = mg.tile([128, CACC, F], FP32, tag="mg")
              nc.sync.dma_start(tmp[:],
                                wrap(bnd[(hop_i, d, "b")], 0, F)[:, :CACC, :])
              nc.vector.tensor_tensor(out=dst, in0=dst, in1=tmp[:],
                                      op=mybir.AluOpType.add)
              if scale_t is None:
                  return
              ts = mg.tile([128, CACC, F], FP32, tag="mg")
              nc.vector.tensor_tensor(out=ts[:], in0=dst, in1=scale_t[:],
                                      op=mybir.AluOpType.mult)
              nc.scalar.dma_start(wrap(bounS[d], 0, F)[:, :CACC, :], ts[:])

          def emit_ag(d):
              nc.gpsimd.collective_compute(
                  "AllGather", mybir.AluOpType.bypass,
                  replica_groups=[list(range(P))],
                  ins=[bounS[d][0:NL6, :].opt()],
                  outs=[ag[d][0:NTAB, :].opt()],
              )

          # hop 1 fwd; AllGather(fwd) is injected a few chunks into hop-1
          # rev so its transfer hides under that hop's SWDGE work
          hop(1, "f", xf_d[0:W, :], xf_d[W:NTAB, :], ES)
          merge_scale(1, "f", 0, doe)
          if stage <= 1:
              raise _StageDone(nc)
          hop(1, "r", xr_d[0:W, :], xr_d[W:NTAB, :], ES,
              hook=lambda: emit_ag("f"))
          merge_scale(1, "r", 1, die)
          if stage <= 2:
              raise _StageDone(nc)

          # --- early: T1 transposes + x/T1 partial gates (run during hop 2)
          for t in range(CACC):
              ps = pt.tile([128, 128], FP32, tag="ps")
              nc.tensor.transpose(out=ps[:], in_=yall[:, t, :],
                                  identity=ident[:])
              nc.vector.tensor_copy(out=phi[0:FO, t * 128:(t + 1) * 128],
                                    in_=ps[0:FO, :])
          for n0 in range(0, NLOCP, NT):
              n1 = min(n0 + NT, NLOCP)
              pxt = px.tile([F, NT], FP32, tag="px")
              nc.sync.dma_start(pxt[:, :n1 - n0], xpt_d[:, n0:n1])
              pgz = pp.tile([FO, NT], FP32, tag="pgz")
              nc.tensor.matmul(out=pgz[:, :n1 - n0], lhsT=wa[0:2 * F, 0:FO],
                               rhs=phi[0:FO, n0:n1], start=True, stop=False)
              nc.tensor.matmul(out=pgz[:, :n1 - n0], lhsT=wx[:, 0:FO],
                               rhs=pxt[:, :n1 - n0], start=False, stop=True)
              nc.vector.tensor_copy(out=gPZ[:, n0:n1], in_=pgz[:, :n1 - n0])
              pgh = pp.tile([FO, NT], FP32, tag="pgh")
              nc.tensor.matmul(out=pgh[:, :n1 - n0], lhsT=wa[0:2 * F, FO:GATES],
                               rhs=phi[0:FO, n0:n1], start=True, stop=False)
              nc.tensor.matmul(out=pgh[:, :n1 - n0], lhsT=wx[:, FO:GATES],
                               rhs=pxt[:, :n1 - n0], start=False, stop=True)
              nc.vector.tensor_copy(out=gPH[:, n0:n1], in_=pgh[:, :n1 - n0])

          # hop 2 fwd (gathers from ag2[f]); AllGather(rev) injected mid-hop
          hop(2, "f", ag["f"][0:W, :], ag["f"][W:NTAB, :], ES,
              hook=lambda: emit_ag("r"))
          merge_scale(2, "f", 2, None)
          # hop 2 rev
          hop(2, "r", ag["r"][0:W, :], ag["r"][W:NTAB, :], ES)
          merge_scale(2, "r", 3, None)
          if stage <= 3:
              raise _StageDone(nc)

          # --- late: Y2 transposes pipelined with gate tiles ----------------
          AF = mybir.ActivationFunctionType
          for n0 in range(0, NLOCP, NT):
              n1 = min(n0 + NT, NLOCP)
              w = n1 - n0
              for t in range(n0 // 128, _ceil(n1, 128)):
                  ps = pt.tile([128, 128], FP32, tag="ps")
                  nc.tensor.transpose(out=ps[:], in_=yall[:, t, :],
                                      identity=ident[:])
                  nc.vector.tensor_copy(
                      out=phi[FO:128, t * 128:(t + 1) * 128],
                      in_=ps[FO:128, :])
              pgz = pp.tile([FO, NT], FP32, tag="pgz")
              nc.tensor.matmul(out=pgz[:, :w], lhsT=wa[2 * F:4 * F, 0:FO],
                               rhs=phi[FO:128, n0:n1], start=True, stop=True)
              gz = px.tile([FO, NT], FP32, tag="px")
              nc.vector.tensor_tensor(out=gz[:, :w], in0=pgz[:, :w],
                                      in1=gPZ[:, n0:n1],
                                      op=mybir.AluOpType.add)
              pgh = pp.tile([FO, NT], FP32, tag="pgh")
              nc.tensor.matmul(out=pgh[:, :w],
                               lhsT=wa[2 * F:4 * F, FO:GATES],
                               rhs=phi[FO:128, n0:n1], start=True, stop=True)
              gh = px.tile([FO, NT], FP32, tag="px")
              nc.vector.tensor_tensor(out=gh[:, :w], in0=pgh[:, :w],
                                      in1=gPH[:, n0:n1],
                                      op=mybir.AluOpType.add)
              nc.scalar.activation(out=gz[:, :w], in_=gz[:, :w],
                                   func=AF.Sigmoid, bias=bS[:], scale=-1.0)
              nc.scalar.activation(out=gh[:, :w], in_=gh[:, :w],
                                   func=AF.Tanh, bias=bT[:], scale=1.0)
              nc.vector.tensor_tensor(out=gz[:, :w], in0=gz[:, :w],
                                      in1=gh[:, :w], op=mybir.AluOpType.mult)
              nc.vector.tensor_scalar_max(gz[:, :w], gz[:, :w], 0.0)
              po = pt.tile([PER, NT], FP32, tag="po")
              nc.tensor.matmul(out=po[:, :w], lhsT=wl[:],
                               rhs=gz[:, :w], start=True, stop=True)
              ot = px.tile([PER, NT], FP32, tag="ot")
              nc.scalar.add(out=ot[:, :w], in_=po[:, :w], add=bl[:])
              nc.scalar.dma_start(out_d[:, n0:n1], ot[:, :w])

      except _StageDone:
          pass
    nc.compile()
    return nc


# ----------------------------------------------------------------------------
# Entry point
# ----------------------------------------------------------------------------

def _assemble(results, meta):
    N, P, NLOC = meta["N"], meta["P"], meta["NLOC"]
    out = np.empty((N, PER), np.float32)
    for p in range(P):
        out[p * NLOC:(p + 1) * NLOC] = results[p]["out"].T[:NLOC]
    return out


def kernel(x, edge_index, edge_weight, w_z, b_z, w_r, b_r, w_h, b_h,
           w_lin, b_lin, _trace=False, _window=WINDOW):
    in_maps, meta = preprocess(x, edge_index, edge_weight, w_z, b_z, w_r,
                               b_r, w_h, b_h, w_lin, b_lin, window=_window)
    nc = build_program(meta)
    res = bass_utils.run_bass_kernel_spmd(
        nc, in_maps, core_ids=list(range(meta["P"])), trace=_trace)
    out = _assemble(res.results, meta)
    if _trace:
        return out, res
    return out

